# revision 1
# baseline (speedup 1.0000x reference)
"""Trainium2 Bass kernel for nn_ChannelSelfAttention (B=4, C=248, T=500, D=64).

Math. The reference computes, per (b, t) slice, channel attention where each
channel scalar x_c is projected by Linear(1, D):
  Q_c = x_c*wq + bq, K_e = x_e*wk + bk, V_e = x_e*wv + bv
  scores[c,e] = <Q_c, K_e>/sqrt(D)
              = (A*x_c*x_e + B1*x_c + G1*x_e + D1)/sqrt(D)
with A = <wq,wk>, B1 = <wq,bk>, G1 = <bq,wk>, D1 = <bq,bk>.
Terms constant along e cancel in softmax, so with s_c = (A*x_c + G1)/sqrt(D):
  wts[c,e] = softmax_e(s_c * x_e)
and since sum_e wts = 1:
  out[b,t,:] = mean_c(sum_e wts[c,e]*x_e) * wv + bv.

Device work per (b, t): broadcast x-row via a rank-1 ones-matmul on the PE
into PSUM, then ScalarE computes exp(s_c*x_e - rowmax_c) with per-partition
scale/bias APs and row-sum accumulation, VectorE normalizes, DMA stores the
[248, 248] wts tile. The tiny `out` tensor is reduced from wts on the host.

Sharding: 8 cores = (batch b = k//2) x (T half = k%2), 250 t's per core.
"""

import numpy as np

import concourse.bass as bass  # noqa: F401  (bass types used via tile/bacc)
import concourse.mybir as mybir
import concourse.tile as tile
from concourse import bacc, bass_utils

B, C, T, D = 4, 248, 500, 64
NCORES = 8
T_CORE = T // 2   # 250 t's per core
GROUP = 50        # x-row staging group size (double buffered)
CH = 124          # partition chunk (2 chunks cover C=248)
F32 = mybir.dt.float32

_NC_CACHE = {}


def build_nc(t_core=T_CORE, group=GROUP, repeats=1):
    """Build the per-core bass program (SPMD: same program, per-core data)."""
    assert t_core % group == 0
    key = (t_core, group, repeats)
    if key in _NC_CACHE:
        return _NC_CACHE[key]

    nc = bacc.Bacc("TRN2", target_bir_lowering=False, debug=False,
                   num_devices=NCORES)

    xrow_d = nc.dram_tensor("xrow", (1, t_core * C), F32, kind="ExternalInput")
    sv_d = nc.dram_tensor("sv", (C, t_core), F32, kind="ExternalInput")
    nrm_d = nc.dram_tensor("nrm", (C, t_core), F32, kind="ExternalInput")
    wts_d = nc.dram_tensor("wts", (t_core, C, C), F32, kind="ExternalOutput")

    with tile.TileContext(nc) as tc:
        with (
            tc.tile_pool(name="const", bufs=1) as cpool,
            tc.tile_pool(name="xstage", bufs=2) as xpool,
            tc.tile_pool(name="psum", bufs=4, space="PSUM") as ppool,
            tc.tile_pool(name="work", bufs=6) as wpool,
            tc.tile_pool(name="dwork", bufs=8) as dpool,
        ):
            ones = cpool.tile([1, CH], F32)
            nc.vector.memset(ones[:], 1.0)

            s_t, n_t = [], []
            for j in range(2):
                s_tile = cpool.tile([CH, t_core], F32, name=f"s_tile{j}")
                nc.sync.dma_start(s_tile[:], sv_d.ap()[j * CH:(j + 1) * CH, :])
                s_t.append(s_tile)
                nrm_tile = cpool.tile([CH, t_core], F32, name=f"nrm_tile{j}")
                nc.sync.dma_start(nrm_tile[:], nrm_d.ap()[j * CH:(j + 1) * CH, :])
                n_t.append(nrm_tile)

            for _rep in range(repeats):
                for g in range(t_core // group):
                    xs = xpool.tile([1, group * C], F32, tag="xs")
                    nc.sync.dma_start(
                        xs[:], xrow_d.ap()[0:1, g * group * C:(g + 1) * group * C]
                    )
                    for i in range(group):
                        t = g * group + i
                        xb = ppool.tile([CH, C], F32, tag="xb")
                        nc.tensor.matmul(
                            xb[:],
                            lhsT=ones[0:1, :],
                            rhs=xs[0:1, i * C:(i + 1) * C],
                            start=True,
                            stop=True,
                        )
                        P = wpool.tile([CH, 2, C], F32, tag="P")
                        den = dpool.tile([CH, 2, 1], F32, tag="den")
                        rcp = dpool.tile([CH, 2, 1], F32, tag="rcp")
                        for j in range(2):
                            nc.scalar.activation(
                                P[:, j, :],
                                xb[:],
                                mybir.ActivationFunctionType.Exp,
                                bias=n_t[j][:, t:t + 1],
                                scale=s_t[j][:, t:t + 1],
                                accum_out=den[:, j, :],
                            )
                        nc.vector.reciprocal(rcp[:], den[:])
                        for j in range(2):
                            nc.vector.tensor_scalar_mul(
                                P[:, j, :], P[:, j, :], rcp[:, j, :]
                            )
                        nc.sync.dma_start(
                            wts_d.ap()[t].rearrange("(j c) e -> c j e", j=2),
                            P[:],
                        )

    nc.compile()
    _NC_CACHE[key] = nc
    return nc


def host_prep(x, wq, bq, wk, bk):
    """s (softmax row scale), -rowmax (stability bias), and row-major x."""
    rD = np.float32(1.0) / np.float32(np.sqrt(np.float32(D)))
    A = np.float32(np.dot(wq, wk))
    G1 = np.float32(np.dot(bq, wk))
    s = (A * x + G1) * rD                       # [B, C, T]
    xmax = x.max(axis=1)                        # [B, T]
    xmin = x.min(axis=1)
    rm = np.maximum(s * xmax[:, None, :], s * xmin[:, None, :])
    xt = np.ascontiguousarray(x.transpose(0, 2, 1))  # [B, T, C]
    return s.astype(np.float32), (-rm).astype(np.float32), xt.astype(np.float32)


def make_in_maps(s, nrm, xt):
    maps = []
    for k in range(NCORES):
        b, t0 = k // 2, (k % 2) * T_CORE
        maps.append({
            "xrow": np.ascontiguousarray(
                xt[b, t0:t0 + T_CORE, :]).reshape(1, T_CORE * C),
            "sv": np.ascontiguousarray(s[b, :, t0:t0 + T_CORE]),
            "nrm": np.ascontiguousarray(nrm[b, :, t0:t0 + T_CORE]),
        })
    return maps


def kernel(x, wq, bq, wk, bk, wv, bv):
    x = np.asarray(x, dtype=np.float32)
    wq, bq, wk, bk, wv, bv = (
        np.asarray(a, dtype=np.float32) for a in (wq, bq, wk, bk, wv, bv)
    )
    s, nrm, xt = host_prep(x, wq, bq, wk, bk)
    nc = build_nc()
    res = bass_utils.run_bass_kernel_spmd(
        nc, make_in_maps(s, nrm, xt), core_ids=list(range(NCORES))
    )

    wts = np.empty((B, T, C, C), dtype=np.float32)
    for k in range(NCORES):
        b, t0 = k // 2, (k % 2) * T_CORE
        wts[b, t0:t0 + T_CORE] = res.results[k]["wts"]

    # out[b,t,:] = mean_c(sum_e wts[b,t,c,e] * x[b,e,t]) * wv + bv
    y = np.matmul(wts, xt[..., None])[..., 0]   # [B, T, C]
    m = y.mean(axis=2)                          # [B, T]
    out = (m[..., None] * wv + bv).astype(np.float32)
    return out, wts


# revision 4
# speedup vs baseline: 576.3313x; 576.3313x over previous
"""Trainium2 Bass kernel for nn_ChannelSelfAttention (B=4, C=248, T=500, D=64).

Math. The reference computes, per (b, t) slice, channel attention where each
channel scalar x_c is projected by Linear(1, D):
  Q_c = x_c*wq + bq, K_e = x_e*wk + bk, V_e = x_e*wv + bv
  scores[c,e] = <Q_c, K_e>/sqrt(D)
              = (A*x_c*x_e + B1*x_c + G1*x_e + D1)/sqrt(D)
with A = <wq,wk>, B1 = <wq,bk>, G1 = <bq,wk>, D1 = <bq,bk>.
Terms constant along e cancel in softmax, so with s_c = (A*x_c + G1)/sqrt(D):
  wts[c,e] = softmax_e(s_c * x_e)
and since sum_e wts = 1:
  out[b,t,:] = mean_c(sum_e wts[c,e]*x_e) * wv + bv.

Device work per (b, t): broadcast x-row via a rank-1 ones-matmul on the PE
into PSUM, then ScalarE computes exp(s_c*x_e - rowmax_c) with per-partition
scale/bias APs and row-sum accumulation, VectorE normalizes, DMA stores the
[248, 248] wts tile. The tiny `out` tensor is reduced from wts on the host.

Sharding: 8 cores = (batch b = k//2) x (T half = k%2), 250 t's per core.
"""

import numpy as np

import concourse.bass as bass  # noqa: F401  (bass types used via tile/bacc)
import concourse.mybir as mybir
import concourse.tile as tile
from concourse import bacc, bass_utils

B, C, T, D = 4, 248, 500, 64
NCORES = 8
T_CORE = T // 2   # 250 t's per core
GROUP = 50        # x-row staging group size (double buffered)
CH = 124          # partition chunk (2 chunks cover C=248)
F32 = mybir.dt.float32

_NC_CACHE = {}


def build_nc(t_core=T_CORE, group=GROUP, loop_r=0):
    """Build the per-core bass program (SPMD: same program, per-core data).

    loop_r=0: normal build, wts is the ExternalOutput.
    loop_r=R>0: timing build — wts is Internal DRAM, the whole t-loop runs R
    times inside a hardware For_i, and a tiny dummy tensor (read back from
    wts) is the only ExternalOutput. This keeps per-call host<->device
    traffic tiny so wall-clock slope over R measures device time.
    """
    assert t_core % group == 0
    key = (t_core, group, loop_r)
    if key in _NC_CACHE:
        return _NC_CACHE[key]

    nc = bacc.Bacc("TRN2", target_bir_lowering=False, debug=False,
                   num_devices=NCORES)

    xrow_d = nc.dram_tensor("xrow", (1, t_core * C), F32, kind="ExternalInput")
    sv_d = nc.dram_tensor("sv", (C, t_core), F32, kind="ExternalInput")
    nrm_d = nc.dram_tensor("nrm", (C, t_core), F32, kind="ExternalInput")
    wts_kind = "ExternalOutput" if loop_r == 0 else "Internal"
    wts_d = nc.dram_tensor("wts", (t_core, C, C), F32, kind=wts_kind)
    dummy_d = None
    if loop_r:
        dummy_d = nc.dram_tensor("tout", (1, 128), F32, kind="ExternalOutput")

    with tile.TileContext(nc) as tc:
        with (
            tc.tile_pool(name="const", bufs=1) as cpool,
            tc.tile_pool(name="xstage", bufs=2) as xpool,
            tc.tile_pool(name="psum", bufs=4, space="PSUM") as ppool,
            tc.tile_pool(name="work", bufs=6) as wpool,
            tc.tile_pool(name="dwork", bufs=8) as dpool,
        ):
            ones = cpool.tile([1, CH], F32)
            nc.vector.memset(ones[:], 1.0)

            s_t, n_t = [], []
            for j in range(2):
                s_tile = cpool.tile([CH, t_core], F32, name=f"s_tile{j}")
                nc.sync.dma_start(s_tile[:], sv_d.ap()[j * CH:(j + 1) * CH, :])
                s_t.append(s_tile)
                nrm_tile = cpool.tile([CH, t_core], F32, name=f"nrm_tile{j}")
                nc.sync.dma_start(nrm_tile[:], nrm_d.ap()[j * CH:(j + 1) * CH, :])
                n_t.append(nrm_tile)

            def body():
                for g in range(t_core // group):
                    xs = xpool.tile([1, group * C], F32, tag="xs", name="xs")
                    nc.sync.dma_start(
                        xs[:], xrow_d.ap()[0:1, g * group * C:(g + 1) * group * C]
                    )
                    for i in range(group):
                        t = g * group + i
                        xb = ppool.tile([CH, C], F32, tag="xb", name="xb")
                        nc.tensor.matmul(
                            xb[:],
                            lhsT=ones[0:1, :],
                            rhs=xs[0:1, i * C:(i + 1) * C],
                            start=True,
                            stop=True,
                        )
                        P = wpool.tile([CH, 2, C], F32, tag="P", name="P")
                        den = dpool.tile([CH, 2, 1], F32, tag="den", name="den")
                        rcp = dpool.tile([CH, 2, 1], F32, tag="rcp", name="rcp")
                        for j in range(2):
                            nc.scalar.activation(
                                P[:, j, :],
                                xb[:],
                                mybir.ActivationFunctionType.Exp,
                                bias=n_t[j][:, t:t + 1],
                                scale=s_t[j][:, t:t + 1],
                                accum_out=den[:, j, :],
                            )
                        nc.vector.reciprocal(rcp[:], den[:])
                        for j in range(2):
                            nc.vector.tensor_scalar_mul(
                                P[:, j, :], P[:, j, :], rcp[:, j, :]
                            )
                        nc.sync.dma_start(
                            wts_d.ap()[t].rearrange("(j c) e -> c j e", j=2),
                            P[:],
                        )

            if loop_r:
                E = mybir.EngineType
                with tc.For_i(0, loop_r, 1,
                              hint_engines=(E.PE, E.Activation, E.DVE, E.SP)):
                    body()
                rb = cpool.tile([1, 128], F32, name="rb")
                nc.sync.dma_start(rb[:], wts_d.ap()[0:1, 0, 0:128])
                nc.sync.dma_start(dummy_d.ap()[:], rb[:])
            else:
                body()

    nc.compile()
    _NC_CACHE[key] = nc
    return nc


def host_prep(x, wq, bq, wk, bk):
    """s (softmax row scale), -rowmax (stability bias), and row-major x."""
    rD = np.float32(1.0) / np.float32(np.sqrt(np.float32(D)))
    A = np.float32(np.dot(wq, wk))
    G1 = np.float32(np.dot(bq, wk))
    s = (A * x + G1) * rD                       # [B, C, T]
    xmax = x.max(axis=1)                        # [B, T]
    xmin = x.min(axis=1)
    rm = np.maximum(s * xmax[:, None, :], s * xmin[:, None, :])
    xt = np.ascontiguousarray(x.transpose(0, 2, 1))  # [B, T, C]
    return s.astype(np.float32), (-rm).astype(np.float32), xt.astype(np.float32)


def make_in_maps(s, nrm, xt):
    maps = []
    for k in range(NCORES):
        b, t0 = k // 2, (k % 2) * T_CORE
        maps.append({
            "xrow": np.ascontiguousarray(
                xt[b, t0:t0 + T_CORE, :]).reshape(1, T_CORE * C),
            "sv": np.ascontiguousarray(s[b, :, t0:t0 + T_CORE]),
            "nrm": np.ascontiguousarray(nrm[b, :, t0:t0 + T_CORE]),
        })
    return maps


def kernel(x, wq, bq, wk, bk, wv, bv):
    x = np.asarray(x, dtype=np.float32)
    wq, bq, wk, bk, wv, bv = (
        np.asarray(a, dtype=np.float32) for a in (wq, bq, wk, bk, wv, bv)
    )
    s, nrm, xt = host_prep(x, wq, bq, wk, bk)
    nc = build_nc()
    res = bass_utils.run_bass_kernel_spmd(
        nc, make_in_maps(s, nrm, xt), core_ids=list(range(NCORES))
    )

    wts = np.empty((B, T, C, C), dtype=np.float32)
    for k in range(NCORES):
        b, t0 = k // 2, (k % 2) * T_CORE
        wts[b, t0:t0 + T_CORE] = res.results[k]["wts"]

    # out[b,t,:] = mean_c(sum_e wts[b,t,c,e] * x[b,e,t]) * wv + bv
    y = np.matmul(wts, xt[..., None])[..., 0]   # [B, T, C]
    m = y.mean(axis=2)                          # [B, T]
    out = (m[..., None] * wv + bv).astype(np.float32)
    return out, wts


# revision 7
# speedup vs baseline: 746.1835x; 1.2947x over previous
"""Trainium2 Bass kernel for nn_ChannelSelfAttention (B=4, C=248, T=500, D=64).

Math. The reference computes, per (b, t) slice, channel attention where each
channel scalar x_c is projected by Linear(1, D):
  Q_c = x_c*wq + bq, K_e = x_e*wk + bk, V_e = x_e*wv + bv
  scores[c,e] = <Q_c, K_e>/sqrt(D)
              = (A*x_c*x_e + B1*x_c + G1*x_e + D1)/sqrt(D)
with A = <wq,wk>, B1 = <wq,bk>, G1 = <bq,wk>, D1 = <bq,bk>.
Terms constant along e cancel in softmax, so with s_c = (A*x_c + G1)/sqrt(D):
  wts[c,e] = softmax_e(s_c * x_e)
and since sum_e wts = 1:
  out[b,t,:] = mean_c(sum_e wts[c,e]*x_e) * wv + bv.

Device work per (b, t): broadcast x-row via a rank-1 ones-matmul on the PE
into PSUM, then ScalarE computes exp(s_c*x_e - rowmax_c) with per-partition
scale/bias APs and row-sum accumulation, VectorE normalizes, DMA stores the
[248, 248] wts tile. The tiny `out` tensor is reduced from wts on the host.

Sharding: 8 cores = (batch b = k//2) x (T half = k%2), 250 t's per core.
"""

import numpy as np

import concourse.bass as bass  # noqa: F401  (bass types used via tile/bacc)
import concourse.mybir as mybir
import concourse.tile as tile
from concourse import bacc, bass_utils

B, C, T, D = 4, 248, 500, 64
NCORES = 8
T_CORE = T // 2   # 250 t's per core
GROUP = 50        # x-row staging group size (double buffered)
GS = 10           # t's per batched wts store
CH = 124          # partition chunk (2 chunks cover C=248)
F32 = mybir.dt.float32

_NC_CACHE = {}


def build_nc(t_core=T_CORE, group=GROUP, gs=GS, loop_r=0):
    """Build the per-core bass program (SPMD: same program, per-core data).

    The per-core wts shard is written t-major ([C, t_core, C]) so each DMA
    descriptor covers gs contiguous t-rows (~10 KB) instead of 992 B; the
    host transposes to [t, c, e] during unshard. Stores are batched gs t's
    per dma_start and alternate between the SP (HWDGE) and GpSimd (SWDGE)
    rings, which measures ~1.3x the single-ring bandwidth here.

    loop_r=0: normal build, wts is the ExternalOutput.
    loop_r=R>0: timing build — wts is Internal DRAM, the whole t-loop runs R
    times inside a hardware For_i, and a tiny dummy tensor (read back from
    wts) is the only ExternalOutput. This keeps per-call host<->device
    traffic tiny so wall-clock slope over R measures device time.
    """
    assert t_core % group == 0 and group % gs == 0
    key = (t_core, group, gs, loop_r)
    if key in _NC_CACHE:
        return _NC_CACHE[key]

    nc = bacc.Bacc("TRN2", target_bir_lowering=False, debug=False,
                   num_devices=NCORES)

    xrow_d = nc.dram_tensor("xrow", (1, t_core * C), F32, kind="ExternalInput")
    sv_d = nc.dram_tensor("sv", (C, t_core), F32, kind="ExternalInput")
    nrm_d = nc.dram_tensor("nrm", (C, t_core), F32, kind="ExternalInput")
    wts_kind = "ExternalOutput" if loop_r == 0 else "Internal"
    wts_d = nc.dram_tensor("wts", (C, t_core, C), F32, kind=wts_kind)
    dummy_d = None
    if loop_r:
        dummy_d = nc.dram_tensor("tout", (1, 128), F32, kind="ExternalOutput")

    with tile.TileContext(nc) as tc:
        with (
            tc.tile_pool(name="const", bufs=1) as cpool,
            tc.tile_pool(name="xstage", bufs=2) as xpool,
            tc.tile_pool(name="psum", bufs=8, space="PSUM") as ppool,
            tc.tile_pool(name="work", bufs=3) as wpool,
            tc.tile_pool(name="dwork", bufs=24) as dpool,
        ):
            ones = cpool.tile([1, CH], F32)
            nc.vector.memset(ones[:], 1.0)

            s_t, n_t = [], []
            for j in range(2):
                s_tile = cpool.tile([CH, t_core], F32, name=f"s_tile{j}")
                nc.sync.dma_start(s_tile[:], sv_d.ap()[j * CH:(j + 1) * CH, :])
                s_t.append(s_tile)
                nrm_tile = cpool.tile([CH, t_core], F32, name=f"nrm_tile{j}")
                nc.sync.dma_start(nrm_tile[:], nrm_d.ap()[j * CH:(j + 1) * CH, :])
                n_t.append(nrm_tile)

            def body():
                for g in range(t_core // group):
                    xs = xpool.tile([1, group * C], F32, tag="xs", name="xs")
                    nc.sync.dma_start(
                        xs[:], xrow_d.ap()[0:1, g * group * C:(g + 1) * group * C]
                    )
                    for bi in range(group // gs):
                        # Pb free layout [j, t, e]: per-(c,j) runs t-contiguous
                        Pb = wpool.tile([CH, 2, gs, C], F32, tag="Pb", name="Pb")
                        for ii in range(gs):
                            t = g * group + bi * gs + ii
                            i = bi * gs + ii
                            xb = ppool.tile([CH, C], F32, tag="xb", name="xb")
                            nc.tensor.matmul(
                                xb[:],
                                lhsT=ones[0:1, :],
                                rhs=xs[0:1, i * C:(i + 1) * C],
                                start=True,
                                stop=True,
                            )
                            den = dpool.tile([CH, 2, 1], F32, tag="den",
                                             name="den")
                            rcp = dpool.tile([CH, 2, 1], F32, tag="rcp",
                                             name="rcp")
                            for j in range(2):
                                nc.scalar.activation(
                                    Pb[:, j, ii, :],
                                    xb[:],
                                    mybir.ActivationFunctionType.Exp,
                                    bias=n_t[j][:, t:t + 1],
                                    scale=s_t[j][:, t:t + 1],
                                    accum_out=den[:, j, :],
                                )
                            nc.vector.reciprocal(rcp[:], den[:])
                            for j in range(2):
                                nc.vector.tensor_scalar_mul(
                                    Pb[:, j, ii, :], Pb[:, j, ii, :],
                                    rcp[:, j, :]
                                )
                        t0 = g * group + bi * gs
                        eng = nc.gpsimd if (t0 // gs) % 2 else nc.sync
                        eng.dma_start(
                            wts_d.ap()[:, t0:t0 + gs, :].rearrange(
                                "(j c) t e -> c j t e", j=2),
                            Pb[:],
                        )

            if loop_r:
                E = mybir.EngineType
                with tc.For_i(0, loop_r, 1,
                              hint_engines=(E.PE, E.Activation, E.DVE, E.SP,
                                            E.Pool)):
                    body()
                rb = cpool.tile([1, 128], F32, name="rb")
                nc.sync.dma_start(rb[:], wts_d.ap()[0:1, 0, 0:128])
                nc.sync.dma_start(dummy_d.ap()[:], rb[:])
            else:
                body()

    nc.compile()
    _NC_CACHE[key] = nc
    return nc


def host_prep(x, wq, bq, wk, bk):
    """s (softmax row scale), -rowmax (stability bias), and row-major x."""
    rD = np.float32(1.0) / np.float32(np.sqrt(np.float32(D)))
    A = np.float32(np.dot(wq, wk))
    G1 = np.float32(np.dot(bq, wk))
    s = (A * x + G1) * rD                       # [B, C, T]
    xmax = x.max(axis=1)                        # [B, T]
    xmin = x.min(axis=1)
    rm = np.maximum(s * xmax[:, None, :], s * xmin[:, None, :])
    xt = np.ascontiguousarray(x.transpose(0, 2, 1))  # [B, T, C]
    return s.astype(np.float32), (-rm).astype(np.float32), xt.astype(np.float32)


def make_in_maps(s, nrm, xt):
    maps = []
    for k in range(NCORES):
        b, t0 = k // 2, (k % 2) * T_CORE
        maps.append({
            "xrow": np.ascontiguousarray(
                xt[b, t0:t0 + T_CORE, :]).reshape(1, T_CORE * C),
            "sv": np.ascontiguousarray(s[b, :, t0:t0 + T_CORE]),
            "nrm": np.ascontiguousarray(nrm[b, :, t0:t0 + T_CORE]),
        })
    return maps


def kernel(x, wq, bq, wk, bk, wv, bv):
    x = np.asarray(x, dtype=np.float32)
    wq, bq, wk, bk, wv, bv = (
        np.asarray(a, dtype=np.float32) for a in (wq, bq, wk, bk, wv, bv)
    )
    s, nrm, xt = host_prep(x, wq, bq, wk, bk)
    nc = build_nc()
    res = bass_utils.run_bass_kernel_spmd(
        nc, make_in_maps(s, nrm, xt), core_ids=list(range(NCORES))
    )

    wts = np.empty((B, T, C, C), dtype=np.float32)
    for k in range(NCORES):
        b, t0 = k // 2, (k % 2) * T_CORE
        # shard is t-major [C, T_CORE, C]; transpose back to [t, c, e]
        wts[b, t0:t0 + T_CORE] = res.results[k]["wts"].transpose(1, 0, 2)

    # out[b,t,:] = mean_c(sum_e wts[b,t,c,e] * x[b,e,t]) * wv + bv
    y = np.matmul(wts, xt[..., None])[..., 0]   # [B, T, C]
    m = y.mean(axis=2)                          # [B, T]
    out = (m[..., None] * wv + bv).astype(np.float32)
    return out, wts
